# revision 3
# baseline (speedup 1.0000x reference)
"""Trainium2 Bass kernel for nn_Encoder: embedding + 2-layer GRU (B=128, S=256,
E=H=1024, V=32000), SPMD across 8 NeuronCores.

Strategy (tensor-parallel over the hidden dim):
  - Host: embedding lookup (pure gather) + per-core weight slicing/transposes.
  - Each core owns a 128-wide slice of H (384 of the 3H gate columns).
  - Per GRU step: PSUM [128B x 512] accumulates
      [ gi_n | r_pre | z_pre | gh_n ]  (cols 0:128 | 128:256 | 256:384 | 384:512)
    via one K=1 bias matmul + 8 input-side matmuls (lhsT = embedded-input
    transposed chunks) + 8 recurrent matmuls (lhsT = gathered h^T chunks),
    all bf16 with fp32 accumulate. Gates on ACT/DVE in fp32. New h-slice is
    PE-transposed, cast to bf16, and exchanged across cores with a per-step
    AllGather collective. Layer 2 runs one step behind layer 1, its
    input-side matmuls reusing layer 1's gathered h^T tiles.
  - Layer-1 input matmul lhsT tiles come from xbar DMA-transpose loads of the
    (device-AllGathered) bf16 embedding matrix.
"""

import os
import sys
import time

for _p in ("/opt/trn_rl_repo", "/root/.axon_site/_ro/trn_rl_repo"):
    if os.path.isdir(_p) and _p not in sys.path:
        sys.path.insert(0, _p)
        break

import numpy as np
import ml_dtypes

import concourse.bass as bass
import concourse.tile as tile
from concourse import bacc, mybir

F32 = mybir.dt.float32
BF16 = mybir.dt.bfloat16

NCORES = 8
VOCAB, EMBED, HIDDEN, LAYERS = 32000, 1024, 1024, 2
BATCH, SEQ = 128, 256
SB = SEQ * BATCH            # 32768 tokens, time-major
SB_C = SB // NCORES         # 4096 tokens gathered per core
HS = HIDDEN // NCORES       # 128   hidden slice
NS = 3 * HS                 # 384   gate-column slice
KC = 8                      # contraction chunks of 128
XBLK = 512                  # tokens per xbar-transposed slab (4 steps)
NSLAB = SB // XBLK          # 64 slabs over the full sequence


def build_nc(n_steps=SEQ):
    nc = bacc.Bacc("TRN2", target_bir_lowering=False, debug=False,
                   num_devices=NCORES)

    x_in = nc.dram_tensor("x", [SB_C, EMBED], BF16, kind="ExternalInput")
    w1t_in = nc.dram_tensor("w1t", [EMBED, NS], BF16, kind="ExternalInput")
    wh0_in = nc.dram_tensor("wh0", [HIDDEN, NS], BF16, kind="ExternalInput")
    w2t_in = nc.dram_tensor("w2t", [HIDDEN, NS], BF16, kind="ExternalInput")
    wh1_in = nc.dram_tensor("wh1", [HIDDEN, NS], BF16, kind="ExternalInput")
    b0_in = nc.dram_tensor("b0", [1, 512], BF16, kind="ExternalInput")
    b1_in = nc.dram_tensor("b1", [1, 512], BF16, kind="ExternalInput")
    id_in = nc.dram_tensor("ident", [128, 128], F32, kind="ExternalInput")

    ys_out = nc.dram_tensor("ys", [SEQ, BATCH, HS], F32, kind="ExternalOutput")
    hf_out = nc.dram_tensor("hfin", [LAYERS, BATCH, HS], F32,
                            kind="ExternalOutput")

    x_bounce = nc.dram_tensor("x_bounce", [SB_C, EMBED], BF16)
    x_full = nc.dram_tensor("x_full", [SB, EMBED], BF16)

    with tile.TileContext(nc) as tc:
        with (
            tc.tile_pool(name="wpool", bufs=1) as wpool,
            tc.tile_pool(name="xslab", bufs=3) as xslab_pool,
            tc.tile_pool(name="ht", bufs=3) as ht_pool,
            tc.tile_pool(name="hprev", bufs=3) as hprev_pool,
            tc.tile_pool(name="gates", bufs=3) as gates_pool,
            tc.tile_pool(name="send", bufs=3) as send_pool,
            tc.tile_pool(name="psum_g", bufs=2, space="PSUM") as psum_g,
            tc.tile_pool(name="psum_tr", bufs=2, space="PSUM") as psum_tr,
            tc.tile_pool(name="dramp", bufs=4, space="DRAM") as dramp,
        ):
            # ---- phase 0: AllGather the embedded sequence across cores ----
            nc.sync.dma_start(x_bounce.ap(), x_in.ap())
            nc.gpsimd.collective_compute(
                "AllGather", mybir.AluOpType.bypass,
                replica_groups=[list(range(NCORES))],
                ins=[x_bounce.ap().opt()],
                outs=[x_full.ap().opt()],
            )

            # ---- phase 1: resident weights / constants ----
            def load_w(name, dram):
                t = wpool.tile([128, KC, NS], BF16, tag=name)
                nc.sync.dma_start(
                    t[:], dram.ap().rearrange("(k p) n -> p k n", p=128))
                return t

            w1t_s = load_w("w1t", w1t_in)
            wh0_s = load_w("wh0", wh0_in)
            w2t_s = load_w("w2t", w2t_in)
            wh1_s = load_w("wh1", wh1_in)

            b0_s = wpool.tile([1, 512], BF16, tag="b0")
            nc.sync.dma_start(b0_s[:], b0_in.ap())
            b1_s = wpool.tile([1, 512], BF16, tag="b1")
            nc.sync.dma_start(b1_s[:], b1_in.ap())
            ones_s = wpool.tile([1, 128], BF16, tag="ones")
            nc.vector.memset(ones_s[:], 1.0)
            ident_s = wpool.tile([128, 128], F32, tag="ident")
            nc.sync.dma_start(ident_s[:], id_in.ap())

            # initial state: zeros
            h1T = ht_pool.tile([128, KC, HS], BF16, tag="h1T")
            nc.vector.memset(h1T[:], 0.0)
            h2T = ht_pool.tile([128, KC, HS], BF16, tag="h2T")
            nc.vector.memset(h2T[:], 0.0)
            h1_prev = hprev_pool.tile([128, HS], F32, tag="h1p")
            nc.vector.memset(h1_prev[:], 0.0)
            h2_prev = hprev_pool.tile([128, HS], F32, tag="h2p")
            nc.vector.memset(h2_prev[:], 0.0)

            xslabs = {}

            def get_xslab(blk):
                if blk not in xslabs:
                    s = xslab_pool.tile([128, KC, XBLK], BF16, tag="xs")
                    for k in range(KC):
                        nc.sync.dma_start_transpose(
                            s[:, k, :],
                            x_full.ap()[blk * XBLK:(blk + 1) * XBLK,
                                        k * 128:(k + 1) * 128])
                    xslabs.clear()
                    xslabs[blk] = s
                return xslabs[blk]

            def gru_step(t, layer, hT_prev, h_prev, gi_lhsT_fn, wi_s, wh_s,
                         bias_s):
                """One GRU step for this core's H-slice. Returns
                (h_new f32 [128,HS], hT_new tile [128,KC,HS] bf16)."""
                ps = psum_g.tile([128, 512], F32, tag=f"ps{layer}")
                # bias row (start=True clears the bank)
                nc.tensor.matmul(ps[:, 0:512], ones_s[:], bias_s[:],
                                 start=True, stop=False)
                # input-side: cols 0:384 = [gi_n | gi_r | gi_z]
                for k in range(KC):
                    nc.tensor.matmul(ps[:, 0:NS], gi_lhsT_fn(k),
                                     wi_s[:, k, :], start=False, stop=False)
                # recurrent: cols 128:512 = [gh_r | gh_z | gh_n]
                for k in range(KC):
                    nc.tensor.matmul(ps[:, 128:512], hT_prev[:, k, :],
                                     wh_s[:, k, :], start=False,
                                     stop=(k == KC - 1))

                rz = gates_pool.tile([128, 2 * HS], F32, tag=f"rz{layer}")
                nc.scalar.activation(rz[:], ps[:, 128:384],
                                     mybir.ActivationFunctionType.Sigmoid)
                t1 = gates_pool.tile([128, HS], F32, tag=f"t1{layer}")
                nc.vector.tensor_mul(t1[:], rz[:, 0:HS], ps[:, 384:512])
                t2 = gates_pool.tile([128, HS], F32, tag=f"t2{layer}")
                nc.vector.tensor_add(t2[:], ps[:, 0:128], t1[:])
                n_t = gates_pool.tile([128, HS], F32, tag=f"n{layer}")
                nc.scalar.activation(n_t[:], t2[:],
                                     mybir.ActivationFunctionType.Tanh)
                hm = gates_pool.tile([128, HS], F32, tag=f"hm{layer}")
                nc.vector.tensor_sub(hm[:], h_prev[:], n_t[:])
                zz = gates_pool.tile([128, HS], F32, tag=f"zz{layer}")
                nc.vector.tensor_mul(zz[:], rz[:, HS:2 * HS], hm[:])
                h_new = hprev_pool.tile([128, HS], F32, tag=f"h{layer+1}p")
                nc.vector.tensor_add(h_new[:], n_t[:], zz[:])

                # transpose + cast for the exchange
                ptr = psum_tr.tile([128, 128], F32, tag="ptr")
                nc.tensor.transpose(ptr[:], h_new[:], ident_s[:])
                snd = send_pool.tile([128, HS], BF16, tag=f"snd{layer}")
                nc.scalar.activation(snd[:], ptr[:],
                                     mybir.ActivationFunctionType.Copy)

                bin_ = dramp.tile([128, HS], BF16, tag=f"bin{layer}")
                bout = dramp.tile([HIDDEN, HS], BF16, tag=f"bout{layer}")
                nc.sync.dma_start(bin_[:], snd[:])
                nc.gpsimd.collective_compute(
                    "AllGather", mybir.AluOpType.bypass,
                    replica_groups=[list(range(NCORES))],
                    ins=[bin_.opt()],
                    outs=[bout.opt()],
                )
                hT_new = ht_pool.tile([128, KC, HS], BF16,
                                      tag=f"h{layer+1}T")
                nc.sync.dma_start(
                    hT_new[:], bout[:].rearrange("(k p) b -> p k b", p=128))
                return h_new, hT_new

            for t in range(n_steps):
                # ---- layer 1, step t ----
                blk, toff = divmod(t * BATCH, XBLK)
                xs = get_xslab(blk)
                h1_new, h1T_new = gru_step(
                    t, 0, h1T, h1_prev,
                    lambda k: xs[:, k, toff:toff + 128],
                    w1t_s, wh0_s, b0_s)

                # ---- layer 2, step t-1 ----
                if t >= 1:
                    h2_new, h2T_new = gru_step(
                        t - 1, 1, h2T, h2_prev,
                        lambda k: h1T[:, k, :],
                        w2t_s, wh1_s, b1_s)
                    nc.scalar.dma_start(ys_out.ap()[t - 1], h2_new[:])
                    h2_prev, h2T = h2_new, h2T_new

                h1_prev, h1T = h1_new, h1T_new

            # ---- epilogue: layer 2, final step ----
            h2_new, h2T_new = gru_step(
                n_steps - 1, 1, h2T, h2_prev,
                lambda k: h1T[:, k, :],
                w2t_s, wh1_s, b1_s)
            nc.scalar.dma_start(ys_out.ap()[n_steps - 1], h2_new[:])
            nc.scalar.dma_start(hf_out.ap()[0], h1_prev[:])
            nc.scalar.dma_start(hf_out.ap()[1], h2_new[:])

    nc.compile()
    return nc


def _prep_inputs(source, emb_W, W_ih, W_hh, b_ih, b_hh):
    """Host-side preprocessing: embedding gather + per-core weight slicing."""
    source = np.asarray(source)
    emb_W = np.asarray(emb_W, dtype=np.float32)
    W_ih = np.asarray(W_ih, dtype=np.float32)
    W_hh = np.asarray(W_hh, dtype=np.float32)
    b_ih = np.asarray(b_ih, dtype=np.float32)
    b_hh = np.asarray(b_hh, dtype=np.float32)

    # time-major token stream: [S, B] -> flat [S*B]
    flat = source.T.reshape(-1)
    X = emb_W[flat].astype(ml_dtypes.bfloat16)  # [SB, E]

    ident = np.eye(128, dtype=np.float32)

    in_maps = []
    for c in range(NCORES):
        r = slice(c * HS, (c + 1) * HS)
        z = slice(HIDDEN + c * HS, HIDDEN + (c + 1) * HS)
        n = slice(2 * HIDDEN + c * HS, 2 * HIDDEN + (c + 1) * HS)

        def wslice(W, order):
            return np.ascontiguousarray(
                np.concatenate([W[o] for o in order], axis=0).T
            ).astype(ml_dtypes.bfloat16)

        def biasrow(l):
            b = np.concatenate([
                b_ih[l][n],                    # gi_n bias
                b_ih[l][r] + b_hh[l][r],       # r pre-act bias
                b_ih[l][z] + b_hh[l][z],       # z pre-act bias
                b_hh[l][n],                    # gh_n bias
            ])
            return b.reshape(1, 512).astype(ml_dtypes.bfloat16)

        in_maps.append({
            "x": X[c * SB_C:(c + 1) * SB_C],
            "w1t": wslice(W_ih[0], [n, r, z]),
            "wh0": wslice(W_hh[0], [r, z, n]),
            "w2t": wslice(W_ih[1], [n, r, z]),
            "wh1": wslice(W_hh[1], [r, z, n]),
            "b0": biasrow(0),
            "b1": biasrow(1),
            "ident": ident,
        })
    return in_maps


_NC_CACHE = {}


def kernel(source, emb_W, W_ih, W_hh, b_ih, b_hh):
    from concourse.bass_utils import run_bass_kernel_spmd

    in_maps = _prep_inputs(source, emb_W, W_ih, W_hh, b_ih, b_hh)

    if "nc" not in _NC_CACHE:
        _NC_CACHE["nc"] = build_nc(SEQ)
    nc = _NC_CACHE["nc"]

    res = run_bass_kernel_spmd(nc, in_maps, core_ids=list(range(NCORES)))

    ys = np.concatenate([res.results[c]["ys"] for c in range(NCORES)], axis=2)
    hf = np.concatenate([res.results[c]["hfin"] for c in range(NCORES)],
                        axis=2)
    return ys.astype(np.float32), hf.astype(np.float32)


# revision 4
# speedup vs baseline: 1.3676x; 1.3676x over previous
"""Trainium2 Bass kernel for nn_Encoder: embedding + 2-layer GRU (B=128, S=256,
E=H=1024, V=32000), SPMD across 8 NeuronCores.

Strategy (tensor-parallel over the hidden dim):
  - Host: embedding lookup (pure gather) + per-core weight slicing/transposes.
  - Each core owns a 128-wide slice of H (384 of the 3H gate columns).
  - Per GRU step: PSUM [128B x 512] accumulates
      [ gi_n | r_pre | z_pre | gh_n ]  (cols 0:128 | 128:256 | 256:384 | 384:512)
    via one K=1 bias matmul + 8 input-side matmuls (lhsT = embedded-input
    transposed chunks) + 8 recurrent matmuls (lhsT = gathered h^T chunks),
    all bf16 with fp32 accumulate. Gates on ACT/DVE in fp32. New h-slice is
    PE-transposed, cast to bf16, and exchanged across cores with a per-step
    AllGather collective. Layer 2 runs one step behind layer 1, its
    input-side matmuls reusing layer 1's gathered h^T tiles.
  - Layer-1 input matmul lhsT tiles come from xbar DMA-transpose loads of the
    (device-AllGathered) bf16 embedding matrix.
"""

import os
import sys
import time

for _p in ("/opt/trn_rl_repo", "/root/.axon_site/_ro/trn_rl_repo"):
    if os.path.isdir(_p) and _p not in sys.path:
        sys.path.insert(0, _p)
        break

import numpy as np
import ml_dtypes

import concourse.bass as bass
import concourse.tile as tile
from concourse import bacc, mybir

F32 = mybir.dt.float32
BF16 = mybir.dt.bfloat16

NCORES = 8
VOCAB, EMBED, HIDDEN, LAYERS = 32000, 1024, 1024, 2
BATCH, SEQ = 128, 256
SB = SEQ * BATCH            # 32768 tokens, time-major
SB_C = SB // NCORES         # 4096 tokens gathered per core
HS = HIDDEN // NCORES       # 128   hidden slice
NS = 3 * HS                 # 384   gate-column slice
KC = 8                      # contraction chunks of 128
XBLK = 512                  # tokens per xbar-transposed slab (4 steps)
NSLAB = SB // XBLK          # 64 slabs over the full sequence


def build_nc(n_steps=SEQ):
    nc = bacc.Bacc("TRN2", target_bir_lowering=False, debug=False,
                   num_devices=NCORES)

    x_in = nc.dram_tensor("x", [SB_C, EMBED], BF16, kind="ExternalInput")
    w1t_in = nc.dram_tensor("w1t", [EMBED, NS], BF16, kind="ExternalInput")
    wh0_in = nc.dram_tensor("wh0", [HIDDEN, NS], BF16, kind="ExternalInput")
    w2t_in = nc.dram_tensor("w2t", [HIDDEN, NS], BF16, kind="ExternalInput")
    wh1_in = nc.dram_tensor("wh1", [HIDDEN, NS], BF16, kind="ExternalInput")
    b0_in = nc.dram_tensor("b0", [1, 512], BF16, kind="ExternalInput")
    b1_in = nc.dram_tensor("b1", [1, 512], BF16, kind="ExternalInput")
    id_in = nc.dram_tensor("ident", [128, 128], F32, kind="ExternalInput")

    ys_out = nc.dram_tensor("ys", [SEQ, BATCH, HS], F32, kind="ExternalOutput")
    hf_out = nc.dram_tensor("hfin", [LAYERS, BATCH, HS], F32,
                            kind="ExternalOutput")

    x_bounce = nc.dram_tensor("x_bounce", [SB_C, EMBED], BF16)
    x_full = nc.dram_tensor("x_full", [SB, EMBED], BF16)

    with tile.TileContext(nc) as tc:
        with (
            tc.tile_pool(name="wpool", bufs=1) as wpool,
            tc.tile_pool(name="xslab", bufs=3) as xslab_pool,
            tc.tile_pool(name="ht", bufs=3) as ht_pool,
            tc.tile_pool(name="hprev", bufs=3) as hprev_pool,
            tc.tile_pool(name="gates", bufs=3) as gates_pool,
            tc.tile_pool(name="send", bufs=3) as send_pool,
            tc.tile_pool(name="psum_g", bufs=2, space="PSUM") as psum_g,
            tc.tile_pool(name="psum_tr", bufs=2, space="PSUM") as psum_tr,
            tc.tile_pool(name="dramp", bufs=4, space="DRAM") as dramp,
        ):
            # ---- phase 0: AllGather the embedded sequence across cores ----
            nc.sync.dma_start(x_bounce.ap(), x_in.ap())
            nc.gpsimd.collective_compute(
                "AllGather", mybir.AluOpType.bypass,
                replica_groups=[list(range(NCORES))],
                ins=[x_bounce.ap().opt()],
                outs=[x_full.ap().opt()],
            )

            # ---- phase 1: resident weights / constants ----
            def load_w(name, dram):
                t = wpool.tile([128, KC, NS], BF16, tag=name)
                nc.sync.dma_start(
                    t[:], dram.ap().rearrange("(k p) n -> p k n", p=128))
                return t

            w1t_s = load_w("w1t", w1t_in)
            wh0_s = load_w("wh0", wh0_in)
            w2t_s = load_w("w2t", w2t_in)
            wh1_s = load_w("wh1", wh1_in)

            b0_s = wpool.tile([1, 512], BF16, tag="b0")
            nc.sync.dma_start(b0_s[:], b0_in.ap())
            b1_s = wpool.tile([1, 512], BF16, tag="b1")
            nc.sync.dma_start(b1_s[:], b1_in.ap())
            ones_s = wpool.tile([1, 128], BF16, tag="ones")
            nc.vector.memset(ones_s[:], 1.0)
            ident_s = wpool.tile([128, 128], F32, tag="ident")
            nc.sync.dma_start(ident_s[:], id_in.ap())

            # initial state: zeros
            h1T = ht_pool.tile([128, KC, HS], BF16, tag="h1T")
            nc.vector.memset(h1T[:], 0.0)
            h2T = ht_pool.tile([128, KC, HS], BF16, tag="h2T")
            nc.vector.memset(h2T[:], 0.0)
            h1_prev = hprev_pool.tile([128, HS], F32, tag="h1p")
            nc.vector.memset(h1_prev[:], 0.0)
            h2_prev = hprev_pool.tile([128, HS], F32, tag="h2p")
            nc.vector.memset(h2_prev[:], 0.0)

            xslabs = {}

            def get_xslab(blk):
                if blk not in xslabs:
                    s = xslab_pool.tile([128, KC, XBLK], BF16, tag="xs")
                    for k in range(KC):
                        nc.sync.dma_start_transpose(
                            s[:, k, :],
                            x_full.ap()[blk * XBLK:(blk + 1) * XBLK,
                                        k * 128:(k + 1) * 128])
                    xslabs.clear()
                    xslabs[blk] = s
                return xslabs[blk]

            def gru_step(t, layer, hT_prev, h_prev, gi_lhsT_fn, wi_s, wh_s,
                         bias_s):
                """One GRU step for this core's H-slice. Returns
                (h_new f32 [128,HS], hT_new tile [128,KC,HS] bf16)."""
                ps = psum_g.tile([128, 512], F32, tag=f"ps{layer}")
                # bias row (start=True clears the bank)
                nc.tensor.matmul(ps[:, 0:512], ones_s[:], bias_s[:],
                                 start=True, stop=False)
                # input-side: cols 0:384 = [gi_n | gi_r | gi_z]
                for k in range(KC):
                    nc.tensor.matmul(ps[:, 0:NS], gi_lhsT_fn(k),
                                     wi_s[:, k, :], start=False, stop=False)
                # recurrent: cols 128:512 = [gh_r | gh_z | gh_n]
                for k in range(KC):
                    nc.tensor.matmul(ps[:, 128:512], hT_prev[:, k, :],
                                     wh_s[:, k, :], start=False,
                                     stop=(k == KC - 1))

                rz = gates_pool.tile([128, 2 * HS], F32, tag=f"rz{layer}")
                nc.scalar.activation(rz[:], ps[:, 128:384],
                                     mybir.ActivationFunctionType.Sigmoid)
                t1 = gates_pool.tile([128, HS], F32, tag=f"t1{layer}")
                nc.vector.tensor_mul(t1[:], rz[:, 0:HS], ps[:, 384:512])
                t2 = gates_pool.tile([128, HS], F32, tag=f"t2{layer}")
                nc.vector.tensor_add(t2[:], ps[:, 0:128], t1[:])
                n_t = gates_pool.tile([128, HS], F32, tag=f"n{layer}")
                nc.scalar.activation(n_t[:], t2[:],
                                     mybir.ActivationFunctionType.Tanh)
                hm = gates_pool.tile([128, HS], F32, tag=f"hm{layer}")
                nc.vector.tensor_sub(hm[:], h_prev[:], n_t[:])
                zz = gates_pool.tile([128, HS], F32, tag=f"zz{layer}")
                nc.vector.tensor_mul(zz[:], rz[:, HS:2 * HS], hm[:])
                h_new = hprev_pool.tile([128, HS], F32, tag=f"h{layer+1}p")
                nc.vector.tensor_add(h_new[:], n_t[:], zz[:])

                # transpose + cast for the exchange
                ptr = psum_tr.tile([128, 128], F32, tag="ptr")
                nc.tensor.transpose(ptr[:], h_new[:], ident_s[:])
                snd = send_pool.tile([128, HS], BF16, tag=f"snd{layer}")
                nc.scalar.activation(snd[:], ptr[:],
                                     mybir.ActivationFunctionType.Copy)

                bin_ = dramp.tile([128, HS], BF16, tag=f"bin{layer}")
                bout = dramp.tile([HIDDEN, HS], BF16, tag=f"bout{layer}")
                nc.sync.dma_start(bin_[:], snd[:])
                nc.gpsimd.collective_compute(
                    "AllGather", mybir.AluOpType.bypass,
                    replica_groups=[list(range(NCORES))],
                    ins=[bin_.opt()],
                    outs=[bout.opt()],
                )
                hT_new = ht_pool.tile([128, KC, HS], BF16,
                                      tag=f"h{layer+1}T")
                nc.sync.dma_start(
                    hT_new[:], bout[:].rearrange("(k p) b -> p k b", p=128))
                return h_new, hT_new

            for t in range(n_steps):
                # ---- layer 1, step t ----
                blk, toff = divmod(t * BATCH, XBLK)
                xs = get_xslab(blk)
                h1_new, h1T_new = gru_step(
                    t, 0, h1T, h1_prev,
                    lambda k: xs[:, k, toff:toff + 128],
                    w1t_s, wh0_s, b0_s)

                # ---- layer 2, step t-1 ----
                if t >= 1:
                    h2_new, h2T_new = gru_step(
                        t - 1, 1, h2T, h2_prev,
                        lambda k: h1T[:, k, :],
                        w2t_s, wh1_s, b1_s)
                    nc.scalar.dma_start(ys_out.ap()[t - 1], h2_new[:])
                    h2_prev, h2T = h2_new, h2T_new

                h1_prev, h1T = h1_new, h1T_new

            # ---- epilogue: layer 2, final step ----
            h2_new, h2T_new = gru_step(
                n_steps - 1, 1, h2T, h2_prev,
                lambda k: h1T[:, k, :],
                w2t_s, wh1_s, b1_s)
            nc.scalar.dma_start(ys_out.ap()[n_steps - 1], h2_new[:])
            nc.scalar.dma_start(hf_out.ap()[0], h1_prev[:])
            nc.scalar.dma_start(hf_out.ap()[1], h2_new[:])

    nc.compile()
    return nc


def _prep_inputs(source, emb_W, W_ih, W_hh, b_ih, b_hh):
    """Host-side preprocessing: embedding gather + per-core weight slicing."""
    source = np.asarray(source)
    emb_W = np.asarray(emb_W, dtype=np.float32)
    W_ih = np.asarray(W_ih, dtype=np.float32)
    W_hh = np.asarray(W_hh, dtype=np.float32)
    b_ih = np.asarray(b_ih, dtype=np.float32)
    b_hh = np.asarray(b_hh, dtype=np.float32)

    # time-major token stream: [S, B] -> flat [S*B]
    flat = source.T.reshape(-1)
    X = emb_W[flat].astype(ml_dtypes.bfloat16)  # [SB, E]

    ident = np.eye(128, dtype=np.float32)

    in_maps = []
    for c in range(NCORES):
        r = slice(c * HS, (c + 1) * HS)
        z = slice(HIDDEN + c * HS, HIDDEN + (c + 1) * HS)
        n = slice(2 * HIDDEN + c * HS, 2 * HIDDEN + (c + 1) * HS)

        def wslice(W, order):
            return np.ascontiguousarray(
                np.concatenate([W[o] for o in order], axis=0).T
            ).astype(ml_dtypes.bfloat16)

        def biasrow(l):
            b = np.concatenate([
                b_ih[l][n],                    # gi_n bias
                b_ih[l][r] + b_hh[l][r],       # r pre-act bias
                b_ih[l][z] + b_hh[l][z],       # z pre-act bias
                b_hh[l][n],                    # gh_n bias
            ])
            return b.reshape(1, 512).astype(ml_dtypes.bfloat16)

        in_maps.append({
            "x": X[c * SB_C:(c + 1) * SB_C],
            "w1t": wslice(W_ih[0], [n, r, z]),
            "wh0": wslice(W_hh[0], [r, z, n]),
            "w2t": wslice(W_ih[1], [n, r, z]),
            "wh1": wslice(W_hh[1], [r, z, n]),
            "b0": biasrow(0),
            "b1": biasrow(1),
            "ident": ident,
        })
    return in_maps


_NC_CACHE = {}


def _get_runner(n_steps=SEQ):
    """Build the bass module once and wrap it in a cached jitted executable
    (run_bass_via_pjrt re-traces and re-lowers on every call otherwise)."""
    key = ("runner", n_steps)
    if key in _NC_CACHE:
        return _NC_CACHE[key]

    import jax
    from jax.sharding import Mesh, PartitionSpec
    from jax.experimental.shard_map import shard_map
    from concourse import bass2jax
    from concourse.bass2jax import _bass_exec_p, partition_id_tensor

    bass2jax.install_neuronx_cc_hook()
    nc = build_nc(n_steps)

    import concourse.mybir as mb
    partition_name = (nc.partition_id_tensor.name
                      if nc.partition_id_tensor else None)
    in_names, out_names, out_avals, zero_shapes = [], [], [], []
    for alloc in nc.m.functions[0].allocations:
        if not isinstance(alloc, mb.MemoryLocationSet):
            continue
        name = alloc.memorylocations[0].name
        if alloc.kind == "ExternalInput":
            if name != partition_name:
                in_names.append(name)
        elif alloc.kind == "ExternalOutput":
            shape = tuple(alloc.tensor_shape)
            dtype = mb.dt.np(alloc.dtype)
            out_names.append(name)
            out_avals.append(jax.core.ShapedArray(shape, dtype))
            zero_shapes.append((shape, dtype))
    n_params = len(in_names)
    all_in = list(in_names) + list(out_names)
    if partition_name is not None:
        all_in.append(partition_name)
    donate = tuple(range(n_params, n_params + len(out_names)))

    def _body(*args):
        operands = list(args)
        if partition_name is not None:
            operands.append(partition_id_tensor())
        outs = _bass_exec_p.bind(
            *operands,
            out_avals=tuple(out_avals),
            in_names=tuple(all_in),
            out_names=tuple(out_names),
            lowering_input_output_aliases=(),
            sim_require_finite=True,
            sim_require_nnan=True,
            nc=nc,
        )
        return tuple(outs)

    devices = jax.devices()[:NCORES]
    mesh = Mesh(np.asarray(devices), ("core",))
    sharded = jax.jit(
        shard_map(_body, mesh=mesh,
                  in_specs=(PartitionSpec("core"),) * (n_params + len(out_names)),
                  out_specs=(PartitionSpec("core"),) * len(out_names),
                  check_rep=False),
        donate_argnums=donate, keep_unused=True)

    def run(in_maps):
        concat_in = [
            np.concatenate([np.asarray(in_maps[c][n]) for c in range(NCORES)],
                           axis=0)
            for n in in_names
        ]
        concat_zeros = [
            np.zeros((NCORES * s[0], *s[1:]), d) for s, d in zero_shapes
        ]
        out_arrs = sharded(*concat_in, *concat_zeros)
        return {
            name: np.asarray(out_arrs[i]).reshape(NCORES, *out_avals[i].shape)
            for i, name in enumerate(out_names)
        }

    _NC_CACHE[key] = run
    return run


def kernel(source, emb_W, W_ih, W_hh, b_ih, b_hh):
    in_maps = _prep_inputs(source, emb_W, W_ih, W_hh, b_ih, b_hh)
    run = _get_runner(SEQ)
    res = run(in_maps)
    ys = np.concatenate([res["ys"][c] for c in range(NCORES)], axis=2)
    hf = np.concatenate([res["hfin"][c] for c in range(NCORES)], axis=2)
    return ys.astype(np.float32), hf.astype(np.float32)


# revision 12
# speedup vs baseline: 2.7508x; 2.0114x over previous
"""Trainium2 Bass kernel for nn_Encoder: embedding + 2-layer GRU (B=128, S=256,
E=H=1024, V=32000), SPMD across 8 NeuronCores.

Strategy (tensor-parallel over the hidden dim):
  - Host: embedding lookup (pure gather) + per-core weight slicing/transposes.
  - Each core owns a 128-wide slice of H (384 of the 3H gate columns).
  - Per GRU step: PSUM [128B x 512] accumulates
      [ gi_n | r_pre | z_pre | gh_n ]  (cols 0:128 | 128:256 | 256:384 | 384:512)
    via one K=1 bias matmul + 8 input-side matmuls (lhsT = embedded-input
    transposed chunks) + 8 recurrent matmuls (lhsT = gathered h^T chunks),
    all bf16 with fp32 accumulate. Gates on ACT/DVE in fp32. New h-slice is
    PE-transposed, cast to bf16, and exchanged across cores with a per-step
    AllGather collective. Layer 2 runs one step behind layer 1, its
    input-side matmuls reusing layer 1's gathered h^T tiles.
  - Layer-1 input matmul lhsT tiles come from xbar DMA-transpose loads of the
    (device-AllGathered) bf16 embedding matrix.
"""

import os
import sys
import time

for _p in ("/opt/trn_rl_repo", "/root/.axon_site/_ro/trn_rl_repo"):
    if os.path.isdir(_p) and _p not in sys.path:
        sys.path.insert(0, _p)
        break

import numpy as np
import ml_dtypes

import concourse.bass as bass
import concourse.tile as tile
from concourse import bacc, mybir

F32 = mybir.dt.float32
BF16 = mybir.dt.bfloat16

NCORES = 8
VOCAB, EMBED, HIDDEN, LAYERS = 32000, 1024, 1024, 2
BATCH, SEQ = 128, 256
SB = SEQ * BATCH            # 32768 tokens, time-major
SB_C = SB // NCORES         # 4096 tokens gathered per core
HS = HIDDEN // NCORES       # 128   hidden slice
NS = 3 * HS                 # 384   gate-column slice
KC = 8                      # contraction chunks of 128
XBLK = 512                  # tokens per xbar-transposed slab (4 steps)
NSLAB = SB // XBLK          # 64 slabs over the full sequence


def build_nc(n_steps=SEQ):
    nc = bacc.Bacc("TRN2", target_bir_lowering=False, debug=False,
                   num_devices=NCORES)

    x_in = nc.dram_tensor("x", [SB_C, EMBED], BF16, kind="ExternalInput")
    w1t_in = nc.dram_tensor("w1t", [EMBED, NS], BF16, kind="ExternalInput")
    wh0_in = nc.dram_tensor("wh0", [HIDDEN, NS], BF16, kind="ExternalInput")
    w2t_in = nc.dram_tensor("w2t", [HIDDEN, NS], BF16, kind="ExternalInput")
    wh1_in = nc.dram_tensor("wh1", [HIDDEN, NS], BF16, kind="ExternalInput")
    b0_in = nc.dram_tensor("b0", [1, 512], BF16, kind="ExternalInput")
    b1_in = nc.dram_tensor("b1", [1, 512], BF16, kind="ExternalInput")
    id_in = nc.dram_tensor("ident", [128, 128], F32, kind="ExternalInput")

    ys_out = nc.dram_tensor("ys", [SEQ, BATCH, HS], BF16,
                            kind="ExternalOutput")
    hf_out = nc.dram_tensor("hfin", [LAYERS, BATCH, HS], F32,
                            kind="ExternalOutput")

    x_bounce = nc.dram_tensor("x_bounce", [SB_C, EMBED], BF16)
    x_full = nc.dram_tensor("x_full", [SB, EMBED], BF16, addr_space="Shared")

    with tile.TileContext(nc) as tc:
        with (
            tc.tile_pool(name="wpool", bufs=1) as wpool,
            tc.tile_pool(name="xslab", bufs=3) as xslab_pool,
            tc.tile_pool(name="ht", bufs=3) as ht_pool,
            tc.tile_pool(name="hprev", bufs=3) as hprev_pool,
            tc.tile_pool(name="gates", bufs=3) as gates_pool,
            tc.tile_pool(name="send", bufs=3) as send_pool,
            tc.tile_pool(name="psum_g", bufs=2, space="PSUM") as psum_g,
            tc.tile_pool(name="psum_tr", bufs=2, space="PSUM") as psum_tr,
            tc.tile_pool(name="dramp", bufs=4, space="DRAM") as dramp,
        ):
            # ---- phase 0: AllGather the embedded sequence across cores ----
            nc.sync.dma_start(x_bounce.ap(), x_in.ap())
            nc.gpsimd.collective_compute(
                "AllGather", mybir.AluOpType.bypass,
                replica_groups=[list(range(NCORES))],
                ins=[x_bounce.ap().opt()],
                outs=[x_full.ap().opt()],
            )

            # ---- phase 1: resident weights / constants ----
            def load_w(name, dram):
                t = wpool.tile([128, KC, NS], BF16, tag=name)
                nc.sync.dma_start(
                    t[:], dram.ap().rearrange("(k p) n -> p k n", p=128))
                return t

            w1t_s = load_w("w1t", w1t_in)
            wh0_s = load_w("wh0", wh0_in)
            w2t_s = load_w("w2t", w2t_in)
            wh1_s = load_w("wh1", wh1_in)

            b0_s = wpool.tile([1, 512], BF16, tag="b0")
            nc.sync.dma_start(b0_s[:], b0_in.ap())
            b1_s = wpool.tile([1, 512], BF16, tag="b1")
            nc.sync.dma_start(b1_s[:], b1_in.ap())
            ones_s = wpool.tile([1, 128], BF16, tag="ones")
            nc.vector.memset(ones_s[:], 1.0)
            ident_s = wpool.tile([128, 128], F32, tag="ident")
            nc.sync.dma_start(ident_s[:], id_in.ap())

            # initial state: zeros
            h1T = ht_pool.tile([128, KC, HS], BF16, tag="h1T")
            nc.vector.memset(h1T[:], 0.0)
            h2T = ht_pool.tile([128, KC, HS], BF16, tag="h2T")
            nc.vector.memset(h2T[:], 0.0)
            h1_prev = hprev_pool.tile([128, HS], F32, tag="h1p")
            nc.vector.memset(h1_prev[:], 0.0)
            h2_prev = hprev_pool.tile([128, HS], F32, tag="h2p")
            nc.vector.memset(h2_prev[:], 0.0)

            xslabs = {}

            def get_xslab(blk):
                if blk not in xslabs:
                    s = xslab_pool.tile([128, KC, XBLK], BF16, tag="xs")
                    for k in range(KC):
                        nc.sync.dma_start_transpose(
                            s[:, k, :],
                            x_full.ap()[blk * XBLK:(blk + 1) * XBLK,
                                        k * 128:(k + 1) * 128])
                    xslabs.clear()
                    xslabs[blk] = s
                return xslabs[blk]

            def gru_step(t, layer, hT_prev, h_prev, gi_lhsT_fn, wi_s, wh_s,
                         bias_s):
                """One GRU step for this core's H-slice. Returns
                (h_new f32 [128,HS], hT_new tile [128,KC,HS] bf16)."""
                ps = psum_g.tile([128, 512], F32, tag=f"ps{layer}")
                # bias row (start=True clears the bank)
                nc.tensor.matmul(ps[:, 0:512], ones_s[:], bias_s[:],
                                 start=True, stop=False)
                # input-side: cols 0:384 = [gi_n | gi_r | gi_z]
                for k in range(KC):
                    nc.tensor.matmul(ps[:, 0:NS], gi_lhsT_fn(k),
                                     wi_s[:, k, :], start=False, stop=False)
                # recurrent: cols 128:512 = [gh_r | gh_z | gh_n]
                for k in range(KC):
                    nc.tensor.matmul(ps[:, 128:512], hT_prev[:, k, :],
                                     wh_s[:, k, :], start=False,
                                     stop=(k == KC - 1))

                rz = gates_pool.tile([128, 2 * HS], F32, tag=f"rz{layer}")
                nc.scalar.activation(rz[:], ps[:, 128:384],
                                     mybir.ActivationFunctionType.Sigmoid)
                t1 = gates_pool.tile([128, HS], F32, tag=f"t1{layer}")
                nc.vector.tensor_mul(t1[:], rz[:, 0:HS], ps[:, 384:512])
                t2 = gates_pool.tile([128, HS], F32, tag=f"t2{layer}")
                nc.vector.tensor_add(t2[:], ps[:, 0:128], t1[:])
                n_t = gates_pool.tile([128, HS], F32, tag=f"n{layer}")
                nc.scalar.activation(n_t[:], t2[:],
                                     mybir.ActivationFunctionType.Tanh)
                hm = gates_pool.tile([128, HS], F32, tag=f"hm{layer}")
                nc.vector.tensor_sub(hm[:], h_prev[:], n_t[:])
                zz = gates_pool.tile([128, HS], F32, tag=f"zz{layer}")
                nc.vector.tensor_mul(zz[:], rz[:, HS:2 * HS], hm[:])
                h_new = hprev_pool.tile([128, HS], F32, tag=f"h{layer+1}p")
                nc.vector.tensor_add(h_new[:], n_t[:], zz[:])

                # transpose + cast for the exchange
                ptr = psum_tr.tile([128, 128], F32, tag="ptr")
                nc.tensor.transpose(ptr[:], h_new[:], ident_s[:])
                snd = send_pool.tile([128, HS], BF16, tag=f"snd{layer}")
                nc.scalar.activation(snd[:], ptr[:],
                                     mybir.ActivationFunctionType.Copy)

                bin_ = dramp.tile([128, HS], BF16, tag=f"bin{layer}")
                bout = dramp.tile([HIDDEN, HS], BF16, tag=f"bout{layer}",
                                  addr_space="Shared")
                nc.sync.dma_start(bin_[:], snd[:])
                nc.gpsimd.collective_compute(
                    "AllGather", mybir.AluOpType.bypass,
                    replica_groups=[list(range(NCORES))],
                    ins=[bin_.opt()],
                    outs=[bout.opt()],
                )
                hT_new = ht_pool.tile([128, KC, HS], BF16,
                                      tag=f"h{layer+1}T")
                nc.sync.dma_start(
                    hT_new[:], bout[:].rearrange("(k p) b -> p k b", p=128))
                return h_new, hT_new

            for t in range(n_steps):
                # ---- layer 1, step t ----
                blk, toff = divmod(t * BATCH, XBLK)
                xs = get_xslab(blk)
                h1_new, h1T_new = gru_step(
                    t, 0, h1T, h1_prev,
                    lambda k: xs[:, k, toff:toff + 128],
                    w1t_s, wh0_s, b0_s)

                # ---- layer 2, step t-1 ----
                if t >= 1:
                    h2_new, h2T_new = gru_step(
                        t - 1, 1, h2T, h2_prev,
                        lambda k: h1T[:, k, :],
                        w2t_s, wh1_s, b1_s)
                    ys_bf = gates_pool.tile([128, HS], BF16, tag="ysbf")
                    nc.scalar.activation(ys_bf[:], h2_new[:],
                                         mybir.ActivationFunctionType.Copy)
                    nc.scalar.dma_start(ys_out.ap()[t - 1], ys_bf[:])
                    h2_prev, h2T = h2_new, h2T_new

                h1_prev, h1T = h1_new, h1T_new

            # ---- epilogue: layer 2, final step ----
            h2_new, h2T_new = gru_step(
                n_steps - 1, 1, h2T, h2_prev,
                lambda k: h1T[:, k, :],
                w2t_s, wh1_s, b1_s)
            ys_bf = gates_pool.tile([128, HS], BF16, tag="ysbf")
            nc.scalar.activation(ys_bf[:], h2_new[:],
                                 mybir.ActivationFunctionType.Copy)
            nc.scalar.dma_start(ys_out.ap()[n_steps - 1], ys_bf[:])
            nc.scalar.dma_start(hf_out.ap()[0], h1_prev[:])
            nc.scalar.dma_start(hf_out.ap()[1], h2_new[:])

    nc.compile()
    return nc


def _prep_inputs(source, emb_W, W_ih, W_hh, b_ih, b_hh):
    """Host-side preprocessing: embedding gather + per-core weight slicing."""
    source = np.asarray(source)
    emb_W = np.asarray(emb_W, dtype=np.float32)
    W_ih = np.asarray(W_ih, dtype=np.float32)
    W_hh = np.asarray(W_hh, dtype=np.float32)
    b_ih = np.asarray(b_ih, dtype=np.float32)
    b_hh = np.asarray(b_hh, dtype=np.float32)

    # time-major token stream: [S, B] -> flat [S*B]
    flat = source.T.reshape(-1)
    X = emb_W[flat].astype(ml_dtypes.bfloat16)  # [SB, E]

    ident = np.eye(128, dtype=np.float32)

    in_maps = []
    for c in range(NCORES):
        r = slice(c * HS, (c + 1) * HS)
        z = slice(HIDDEN + c * HS, HIDDEN + (c + 1) * HS)
        n = slice(2 * HIDDEN + c * HS, 2 * HIDDEN + (c + 1) * HS)

        def wslice(W, order):
            return np.ascontiguousarray(
                np.concatenate([W[o] for o in order], axis=0).T
            ).astype(ml_dtypes.bfloat16)

        def biasrow(l):
            b = np.concatenate([
                b_ih[l][n],                    # gi_n bias
                b_ih[l][r] + b_hh[l][r],       # r pre-act bias
                b_ih[l][z] + b_hh[l][z],       # z pre-act bias
                b_hh[l][n],                    # gh_n bias
            ])
            return b.reshape(1, 512).astype(ml_dtypes.bfloat16)

        in_maps.append({
            "x": X[c * SB_C:(c + 1) * SB_C],
            "w1t": wslice(W_ih[0], [n, r, z]),
            "wh0": wslice(W_hh[0], [r, z, n]),
            "w2t": wslice(W_ih[1], [n, r, z]),
            "wh1": wslice(W_hh[1], [r, z, n]),
            "b0": biasrow(0),
            "b1": biasrow(1),
            "ident": ident,
        })
    return in_maps


_NC_CACHE = {}


def _get_runner(n_steps=SEQ):
    """Build the bass module once and wrap it in a cached jitted executable
    (run_bass_via_pjrt re-traces and re-lowers on every call otherwise)."""
    key = ("runner", n_steps)
    if key in _NC_CACHE:
        return _NC_CACHE[key]

    import jax
    from jax.sharding import Mesh, PartitionSpec
    from jax.experimental.shard_map import shard_map
    from concourse import bass2jax
    from concourse.bass2jax import _bass_exec_p, partition_id_tensor

    bass2jax.install_neuronx_cc_hook()
    nc = build_nc(n_steps)

    import concourse.mybir as mb
    partition_name = (nc.partition_id_tensor.name
                      if nc.partition_id_tensor else None)
    in_names, out_names, out_avals, zero_shapes = [], [], [], []
    for alloc in nc.m.functions[0].allocations:
        if not isinstance(alloc, mb.MemoryLocationSet):
            continue
        name = alloc.memorylocations[0].name
        if alloc.kind == "ExternalInput":
            if name != partition_name:
                in_names.append(name)
        elif alloc.kind == "ExternalOutput":
            shape = tuple(alloc.tensor_shape)
            dtype = mb.dt.np(alloc.dtype)
            out_names.append(name)
            out_avals.append(jax.core.ShapedArray(shape, dtype))
            zero_shapes.append((shape, dtype))
    n_params = len(in_names)
    all_in = list(in_names) + list(out_names)
    if partition_name is not None:
        all_in.append(partition_name)
    donate = tuple(range(n_params, n_params + len(out_names)))

    import jax.numpy as jnp
    from jax.sharding import NamedSharding

    def _body(*args):
        operands = list(args)
        if partition_name is not None:
            operands.append(partition_id_tensor())
        outs = _bass_exec_p.bind(
            *operands,
            out_avals=tuple(out_avals),
            in_names=tuple(all_in),
            out_names=tuple(out_names),
            lowering_input_output_aliases=(),
            sim_require_finite=True,
            sim_require_nnan=True,
            nc=nc,
        )
        return tuple(outs)

    devices = jax.devices()[:NCORES]
    mesh = Mesh(np.asarray(devices), ("core",))
    sharded = jax.jit(
        shard_map(_body, mesh=mesh,
                  in_specs=(PartitionSpec("core"),) * (n_params + len(out_names)),
                  out_specs=(PartitionSpec("core"),) * len(out_names),
                  check_rep=False),
        donate_argnums=donate, keep_unused=True)

    # device-side creation of the donated output backing buffers
    zeros_sharding = NamedSharding(mesh, PartitionSpec("core"))
    make_zeros = jax.jit(
        lambda: tuple(jnp.zeros((NCORES * s[0], *s[1:]), d)
                      for s, d in zero_shapes),
        out_shardings=(zeros_sharding,) * len(zero_shapes))

    def run(in_maps):
        concat_in = [
            np.concatenate([np.asarray(in_maps[c][n]) for c in range(NCORES)],
                           axis=0)
            for n in in_names
        ]
        out_arrs = sharded(*concat_in, *make_zeros())
        return {
            name: np.asarray(out_arrs[i]).reshape(NCORES, *out_avals[i].shape)
            for i, name in enumerate(out_names)
        }

    _NC_CACHE[key] = run
    return run


def kernel(source, emb_W, W_ih, W_hh, b_ih, b_hh):
    in_maps = _prep_inputs(source, emb_W, W_ih, W_hh, b_ih, b_hh)
    run = _get_runner(SEQ)
    res = run(in_maps)
    ys = np.concatenate([res["ys"][c].astype(np.float32)
                         for c in range(NCORES)], axis=2)
    hf = np.concatenate([res["hfin"][c].astype(np.float32)
                         for c in range(NCORES)], axis=2)
    return ys, hf
